# revision 50
# baseline (speedup 1.0000x reference)
"""Causal self-attention (B=4, S=2048, D=768, H=12) on 8 trn2 NeuronCores.

Sharding: core c -> (batch b = c//2, head-half hh = c%2). Each core handles
one batch and 6 of the 12 heads: it computes qkv for its 384 q/k/v columns,
full causal attention for its 6 heads, and a partial output projection over
its 384 rows of w_proj. Host sums the two half partials per batch + b_proj.

Device pipeline (bf16 matmul operands / f32 PSUM accumulation):
  xT 6x[128,2048] = x^T via PE transposes; QT/KT 3x[128,2048] pack 2 heads per
  128 partitions (q pre-scaled by 1/8); VV 16x[128,390] are v s-chunk tiles
  with a ones column per head so A@V also yields the softmax rowsum.
  Attention runs per (q-chunk c of 512) x (head-pack t): both heads' S^T
  strips (k on partitions, q on free dim) are computed concurrently via PE
  row-tiling into one [128,1024] PSUM tile (head A rows 0-63 -> bank 0, head
  B rows 64-127 -> bank 1); one ScalarE exp per strip-pair reads the valid
  columns through a 3D access pattern; causal mask = 0/1 upper-triangular
  multiply on the diagonal block only; U~^T = V~^T @ expS^T accumulates in
  [65,512] PSUM per head (row 64 = rowsum). After each (c,t): extract U^T +
  rowsum, replicate rowsum across partitions (K=1 matmul), reciprocal, and
  normalize U^T in place; after each c: partial projection for its 4
  s-chunks + output DMA. A single 2-bank tag-shared PSUM pool serves
  qkv/transpose/replicate/proj so attention's 6 banks stay disjoint and all
  phases overlap.
"""

import numpy as np

B, S, D, H, HD = 4, 2048, 768, 12, 64
HPC = 6  # heads per core
N_CORES = 8

_built_nc = None


def _build():
    import concourse.bass as bass
    import concourse.mybir as mybir
    from concourse import bacc
    import concourse.tile as tile
    from concourse.masks import make_identity, make_upper_triangular
    from contextlib import ExitStack

    f32 = mybir.dt.float32
    bf16 = mybir.dt.bfloat16
    FT = mybir.ActivationFunctionType
    MUL = mybir.AluOpType.mult

    nc = bacc.Bacc("TRN2", target_bir_lowering=False, debug=False)
    # x/w/wp arrive pre-cast to bf16 from the host: halves HBM input traffic
    x_d = nc.dram_tensor("x_in", [S, D], bf16, kind="ExternalInput").ap()
    w_d = nc.dram_tensor("w_in", [D, 1152], bf16, kind="ExternalInput").ap()
    bqkv_d = nc.dram_tensor("bqkv_in", [1152], f32, kind="ExternalInput").ap()
    wp_d = nc.dram_tensor("wp_in", [384, D], bf16, kind="ExternalInput").ap()
    out_d = nc.dram_tensor("out", [S, D], f32, kind="ExternalOutput").ap()

    with tile.TileContext(nc) as tc, ExitStack() as ctx:
        # ---------------- constants + persistent tiles ----------------
        pconst = ctx.enter_context(tc.tile_pool(name="const", bufs=1))
        ident = pconst.tile([128, 128], bf16)
        make_identity(nc, ident[:])
        utri = pconst.tile([128, 128], bf16)  # 1.0 where p <= c else 0.0
        make_upper_triangular(nc, utri[:], val=1.0, diag=True)
        ones1 = pconst.tile([1, 128], bf16)
        nc.vector.memset(ones1[:], 1.0)
        ones33 = pconst.tile([33, 128], bf16)  # ones rows at 32-aligned bases
        nc.vector.memset(ones33[:], 1.0)
        bq = pconst.tile([128, 6], f32)  # per-chunk bias vecs: cols 0-2 q, 3-5 k
        nc.sync.dma_start(bq[:], bqkv_d[0:768].rearrange("(c p) -> p c", p=128))
        bv_row = pconst.tile([1, 384], bf16)
        nc.gpsimd.dma_start(bv_row[:], bqkv_d[768:1152].rearrange("(o n) -> o n", o=1))
        bvb = pconst.tile([128, 384], f32)  # bias_v broadcast to 128 partitions

        pqkv = ctx.enter_context(tc.tile_pool(name="qkvout", bufs=1))
        # QT/KT split per 512-col s-chunk so attention chunk c only depends on
        # the matching qkv chunk (Tile deps are tile-granular)
        QT = [
            [pqkv.tile([128, 512], bf16, name=f"qt{t}_{sc}") for sc in range(4)]
            for t in range(3)
        ]
        KT = [
            [pqkv.tile([128, 512], bf16, name=f"kt{t}_{sc}") for sc in range(4)]
            for t in range(3)
        ]
        VV = [pqkv.tile([128, HPC * 65], bf16, name=f"vv{i}") for i in range(16)]
        # pack t's bf16 rowsums: head 2t -> row 0, head 2t+1 -> row 32
        RCP = [pqkv.tile([33, S], bf16, name=f"rcp{t}") for t in range(3)]
        UT = [pqkv.tile([128, S], bf16, name=f"ut{t}") for t in range(3)]
        wpt = pqkv.tile([128, 3, D], bf16)
        nc.sync.dma_start(wpt[:], wp_d.rearrange("(c p) n -> p c n", p=128))
        pes = ctx.enter_context(tc.tile_pool(name="espool", bufs=6))
        pnrm = ctx.enter_context(tc.tile_pool(name="nrm", bufs=3))
        pout = ctx.enter_context(tc.tile_pool(name="outp", bufs=4))

        # attention PSUM (6 banks) + shared 1-bank-slot pool (2 banks)
        pst2 = ctx.enter_context(tc.tile_pool(name="stps", space="PSUM", bufs=2))
        pav = ctx.enter_context(tc.tile_pool(name="avps", space="PSUM", bufs=1))
        pmm = ctx.enter_context(tc.tile_pool(name="mmps", space="PSUM", bufs=2))

        # ------- interleaved: per 512-chunk qkv production + attention -------
        p1 = ctx.enter_context(tc.tile_pool(name="ph1", bufs=1))
        pxs = ctx.enter_context(tc.tile_pool(name="xstage", bufs=4))
        wt = p1.tile([128, 6, 1152], bf16)
        nc.sync.dma_start(wt[:], w_d.rearrange("(c p) n -> p c n", p=128))
        xT = [
            [p1.tile([128, 512], bf16, name=f"xt{c}_{sc}") for sc in range(4)]
            for c in range(6)
        ]

        # broadcast v-bias (K=1 bf16 matmul against a ones row)
        psb = pmm.tile([128, 384], f32, tag="mm")
        nc.tensor.matmul(psb[:], lhsT=ones1[:], rhs=bv_row[:])
        nc.vector.tensor_copy(bvb[:], psb[:])

        for sc in range(4):
            # -- qkv chunk sc: load + transpose + V + QT/KT --
            if True:
                i0 = sc * 4
                # one grouped 790KB DMA per chunk (small per-tile DMAs are
                # descriptor-dominated below the ~1MB knee)
                xg = pxs.tile([128, 4, D], bf16, tag="xs", name=f"xg{sc}")
                nc.sync.dma_start(
                    xg[:],
                    x_d[sc * 512 : (sc + 1) * 512, :].rearrange(
                        "(i p) d -> p i d", p=128
                    ),
                )
                xs4 = [xg[:, di, :] for di in range(4)]
                for c in range(6):
                    pt = pmm.tile([128, 512], bf16, tag="mm")
                    for di in range(4):
                        nc.tensor.transpose(
                            pt[:, di * 128 : (di + 1) * 128],
                            xs4[di][:, c * 128 : (c + 1) * 128],
                            ident[:],
                        )
                    nc.vector.tensor_copy(xT[c][sc][:], pt[:])
                for i in range(i0, i0 + 4):
                    psv = pmm.tile([128, 384], f32, tag="mm")
                    for c in range(6):
                        nc.tensor.matmul(
                            psv[:],
                            lhsT=xT[c][sc][:, (i - i0) * 128 : (i - i0 + 1) * 128],
                            rhs=wt[:, c, 768:1152],
                            start=(c == 0),
                            stop=(c == 5),
                        )
                    vt = VV[i][:].rearrange("p (h m) -> p h m", m=65)
                    nc.vector.tensor_tensor(
                        vt[:, :, 0:64],
                        psv[:].rearrange("p (h m) -> p h m", m=64),
                        bvb[:].rearrange("p (h m) -> p h m", m=64),
                        mybir.AluOpType.add,
                    )
                    nc.vector.memset(vt[:, :, 64:65], 1.0)
                for ncI in range(3):
                    for which, dst in ((0, QT), (1, KT)):
                        base = which * 384
                        ps = pmm.tile([128, 512], f32, tag="mm")
                        for c in range(6):
                            nc.tensor.matmul(
                                ps[:],
                                lhsT=wt[:, c, base + ncI * 128 : base + (ncI + 1) * 128],
                                rhs=xT[c][sc][:],
                                start=(c == 0),
                                stop=(c == 5),
                            )
                        cidx = which * 3 + ncI
                        # drain on ScalarE: DVE chokes at chunk handoffs while
                        # ACT is idle exactly there
                        nc.scalar.activation(
                            dst[ncI][sc][:],
                            ps[:],
                            FT.Identity,
                            bias=bq[:, cidx : cidx + 1],
                        )

            # -- attention / norm / projection for chunk c == sc --
            c = sc
            g0 = c * 512  # global q base of this chunk
            for t in range(3):
                avA = pav.tile([65, 512], f32, tag="avA")
                avB = pav.tile([65, 512], f32, tag="avB")
                for j in range(4 * c + 4):
                    n0 = max(0, j * 128 - g0)
                    W = 512 - n0
                    jc, jr = j // 4, (j % 4) * 128
                    # ScalarE's exp stream is the kernel bottleneck: schedule
                    # the score matmuls + exp at max priority so PE keeps the
                    # ACT queue fed; qkv/AV/proj matmuls fill PE gaps
                    with tc.high_priority():
                        st = pst2.tile([128, 1024], f32, tag="st")
                        # both heads' strips concurrently (PE rows 0-63/64-127)
                        nc.tensor.matmul(
                            st[:, 0:W],
                            lhsT=KT[t][jc][0:64, jr : jr + 128],
                            rhs=QT[t][c][0:64, n0:512],
                            start=True,
                            stop=True,
                        )
                        nc.tensor.matmul(
                            st[:, 512 : 512 + W],
                            lhsT=KT[t][jc][64:128, jr : jr + 128],
                            rhs=QT[t][c][64:128, n0:512],
                            start=True,
                            stop=True,
                        )
                        es = pes.tile([128, 1024], bf16, tag="es")
                        nc.scalar.activation(
                            es[:].rearrange("p (h w) -> p h w", h=2)[:, :, 0:W],
                            st[:].rearrange("p (h w) -> p h w", h=2)[:, :, 0:W],
                            FT.Exp,
                        )
                    if j * 128 >= g0:  # diagonal block at start of valid region
                        nc.gpsimd.tensor_tensor(
                            es[:, 0:128], es[:, 0:128], utri[:], MUL
                        )
                        nc.gpsimd.tensor_tensor(
                            es[:, 512:640], es[:, 512:640], utri[:], MUL
                        )
                    last = j == 4 * c + 3
                    nc.tensor.matmul(
                        avA[:, n0:512],
                        lhsT=VV[j][:, (2 * t) * 65 : (2 * t + 1) * 65],
                        rhs=es[:, 0:W],
                        start=(j == 0),
                        stop=last,
                    )
                    nc.tensor.matmul(
                        avB[:, n0:512],
                        lhsT=VV[j][:, (2 * t + 1) * 65 : (2 * t + 2) * 65],
                        rhs=es[:, 512 : 512 + W],
                        start=(j == 0),
                        stop=last,
                    )
                # extract U^T + rowsums
                nc.vector.tensor_copy(UT[t][0:64, g0 : g0 + 512], avA[0:64, :])
                nc.vector.tensor_copy(UT[t][64:128, g0 : g0 + 512], avB[0:64, :])
                nc.vector.tensor_copy(RCP[t][0:1, g0 : g0 + 512], avA[64:65, :])
                nc.vector.tensor_copy(RCP[t][32:33, g0 : g0 + 512], avB[64:65, :])
                # normalize: replicate rowsums across partitions, recip, multiply
                # (st slots are idle between attention chunks; using them keeps
                # the mm pool free for next-chunk qkv groups. For the LAST
                # chunk qkv is done, so use the mm pool instead -- the st slots
                # are still busy with the final pack's strips and this lets the
                # tail projection overlap them)
                prpool, prtag = (pmm, "mm") if c == 3 else (pst2, "st")
                pr = prpool.tile([128, 512], f32, tag=prtag)
                nc.tensor.matmul(
                    pr[0:64, :],
                    lhsT=ones33[0:1, 0:64],
                    rhs=RCP[t][0:1, g0 : g0 + 512],
                )
                nc.tensor.matmul(
                    pr[64:128, :],
                    lhsT=ones33[32:33, 0:64],
                    rhs=RCP[t][32:33, g0 : g0 + 512],
                )
                rec = pnrm.tile([128, 512], f32, tag="rec")
                nc.vector.reciprocal_approx_fast(rec[:], pr[:])
                nc.vector.tensor_tensor(
                    UT[t][:, g0 : g0 + 512], UT[t][:, g0 : g0 + 512], rec[:], MUL
                )
            # partial projection + store for this chunk's 4 s-tiles
            for i in range(4 * c, 4 * c + 4):
                poa = prpool.tile([128, 512], f32, tag=prtag)
                pob = prpool.tile([128, 256], f32, tag=prtag)
                for t in range(3):
                    nc.tensor.matmul(
                        poa[:],
                        lhsT=UT[t][:, i * 128 : (i + 1) * 128],
                        rhs=wpt[:, t, 0:512],
                        start=(t == 0),
                        stop=(t == 2),
                    )
                for t in range(3):
                    nc.tensor.matmul(
                        pob[:],
                        lhsT=UT[t][:, i * 128 : (i + 1) * 128],
                        rhs=wpt[:, t, 512:768],
                        start=(t == 0),
                        stop=(t == 2),
                    )
                ob = pout.tile([128, D], f32, tag="ob")
                nc.scalar.copy(ob[:, 0:512], poa[:])
                nc.scalar.copy(ob[:, 512:768], pob[:])
                nc.sync.dma_start(out_d[i * 128 : (i + 1) * 128, :], ob[:])

    nc.compile()
    return nc


def _get_nc():
    global _built_nc
    if _built_nc is None:
        _built_nc = _build()
    return _built_nc


def _make_in_maps(x, w_qkv, b_qkv, w_proj):
    import ml_dtypes

    bf16 = ml_dtypes.bfloat16
    in_maps = []
    xb = [np.ascontiguousarray(x[b].astype(bf16)) for b in range(B)]
    for core in range(N_CORES):
        b, hh = core // 2, core % 2
        cs = slice(hh * 384, (hh + 1) * 384)
        wq = w_qkv[:, 0:768][:, cs] * np.float32(0.125)  # fold 1/sqrt(64)
        wk = w_qkv[:, 768:1536][:, cs]
        wv = w_qkv[:, 1536:2304][:, cs]
        w_in = np.ascontiguousarray(
            np.concatenate([wq, wk, wv], axis=1).astype(bf16)
        )
        bqv = np.concatenate(
            [
                b_qkv[0:768][cs] * np.float32(0.125),
                b_qkv[768:1536][cs],
                b_qkv[1536:2304][cs],
            ]
        ).astype(np.float32)
        wp = np.ascontiguousarray(w_proj[cs, :].astype(bf16))
        in_maps.append(
            {
                "x_in": xb[b],
                "w_in": w_in,
                "bqkv_in": bqv,
                "wp_in": wp,
            }
        )
    return in_maps


def _run(x, w_qkv, b_qkv, w_proj, b_proj, trace=False):
    from concourse.bass_utils import run_bass_kernel_spmd

    nc = _get_nc()
    in_maps = _make_in_maps(x, w_qkv, b_qkv, w_proj)
    res = run_bass_kernel_spmd(
        nc, in_maps, core_ids=list(range(N_CORES)), trace=trace
    )
    out = np.zeros((B, S, D), np.float32)
    for core in range(N_CORES):
        out[core // 2] += res.results[core]["out"]
    out += np.asarray(b_proj, np.float32)[None, None, :]
    return out, res


def kernel(**inputs):
    x = np.asarray(inputs["x"], np.float32)
    w_qkv = np.asarray(inputs["w_qkv"], np.float32)
    b_qkv = np.asarray(inputs["b_qkv"], np.float32)
    w_proj = np.asarray(inputs["w_proj"], np.float32)
    b_proj = np.asarray(inputs["b_proj"], np.float32)
    out, _ = _run(x, w_qkv, b_qkv, w_proj, b_proj, trace=False)
    return out
